# revision 8
# baseline (speedup 1.0000x reference)
# kernel.py — nn_CustomLinearEval: group-dequantized linear layer on 8 trn2 cores.
#
# out[b,s,n] = sum_k x[b,s,k] * w_dq[k,n] + bias[n]
#   w_dq = round(weight.T / s) * s,  s = step_scales[g,n] + 1e-8, g = k // 128
#
# Sharding: column-parallel (tensor-parallel over N). Each core owns 512 of the
# 4096 output features:
#   - DMAs its [512, 4096] fp32 weight shard on the gpsimd (SWDGE) queue so it
#     doesn't contend with the x stream; dequantizes in natural [n, k] layout
#     with round-half-even via the +/-1.5*2^23 magic trick (matching
#     jnp.round): scalar engine does t1 = w*(1/s) + MAGIC (its native
#     multiply-add form, exact), DVE does w_dq = (t1 - MAGIC)*s (subtract
#     FIRST: doing s*t1 - MAGIC*s instead catastrophically cancels).
#     The 128 [n,k] tiles are transposed on the PE once; w_dq^T (bf16, 4 MiB)
#     stays SBUF-resident.
#   - Streams host-pre-transposed bf16 x^T [K, M] in m-blocks of
#     [512,512,512,1024*6,512] columns (double-buffered; chunk DMAs alternate
#     between the sync and scalar HWDGE queues), running back-to-back bf16
#     matmuls (free dim 512) accumulating out^T in PSUM over 32 k-tiles.
#     1024-wide blocks share one LDWEIGHTS per two matmuls.
#   - Block-0 AND block-1 matmul passes are interleaved with the phase-0 rows
#     in PE program order, so the PE has ~14us of matmul work per ~8us
#     dequant row and never idles during warmup.
#   - Bias-add fuses into the PSUM->SBUF copy on the scalar engine; out DMAs
#     issue from the scalar engine's DGE.
# Host gathers the 8 out^T row-shards and transposes once in numpy.

import numpy as np
import ml_dtypes

BF16 = ml_dtypes.bfloat16

GS = 128
EPS = 1e-8
B, S, K, N = 4, 2048, 4096, 4096
M = B * S
NCORES = 8
NL = N // NCORES          # 512 out-features per core
G = K // GS               # 32 quant groups
NT = NL // 128            # 4 n tiles per core
KT = K // 128             # 32 k tiles
MBS = [512, 512, 512] + [1024] * 6 + [512]   # m-block column ramp (sums to M)
MOFF = [sum(MBS[:i]) for i in range(len(MBS))]
MAGIC = float(np.float32(12582912.0))  # 1.5 * 2**23: fp32 round-to-nearest-even

_NC_CACHE = {}


def _build_nc():
    import concourse.bass as bass
    import concourse.mybir as mybir
    import concourse.tile as tile

    f32 = mybir.dt.float32
    bf16 = mybir.dt.bfloat16
    AF = mybir.ActivationFunctionType
    OP = mybir.AluOpType

    nc = bass.Bass()
    # x_t: host-pre-transposed x, [K, M] bf16 (pure layout transform on host)
    x_t = nc.dram_tensor("x_t", [K, M], bf16, kind="ExternalInput")
    w = nc.dram_tensor("w", [NL, K], f32, kind="ExternalInput")
    srep = nc.dram_tensor("srep", [128, NT * G], f32, kind="ExternalInput")
    rrep = nc.dram_tensor("rrep", [128, NT * G], f32, kind="ExternalInput")
    brep = nc.dram_tensor("brep", [128, NT], f32, kind="ExternalInput")
    ident = nc.dram_tensor("ident", [128, 128], bf16, kind="ExternalInput")
    out_t = nc.dram_tensor("out_t", [NL, M], f32, kind="ExternalOutput")

    WCH = 2048                # k-columns per weight DMA chunk (16 k-tiles)
    NGRP = KT // 4            # 8 transpose groups of 4 k-tiles per n row
    XMAX = KT * 1024          # x block tile columns (max block)

    with tile.TileContext(nc) as tc:
        with (
            tc.tile_pool(name="const", bufs=1) as constp,
            tc.tile_pool(name="wdqT", bufs=1) as wdqTp,
            tc.tile_pool(name="xblk", bufs=2) as xp,
            tc.tile_pool(name="wnat", bufs=2) as wnatp,
            tc.tile_pool(name="t1", bufs=4) as t1p,
            tc.tile_pool(name="wdq", bufs=8) as wdqp,
            tc.tile_pool(name="outsb", bufs=3) as outp,
            tc.tile_pool(name="tp_ps", bufs=2, space="PSUM") as tpps,
            tc.tile_pool(name="acc_ps", bufs=3, space="PSUM") as accps,
        ):
            # consts ride the gpsimd queue ahead of the weight chunks
            id_sb = constp.tile([128, 128], bf16)
            nc.gpsimd.dma_start(id_sb[:], ident[:, :])
            s_sb = constp.tile([128, NT * G], f32)
            nc.gpsimd.dma_start(s_sb[:], srep[:, :])
            r_sb = constp.tile([128, NT * G], f32)
            nc.gpsimd.dma_start(r_sb[:], rrep[:, :])
            b_sb = constp.tile([128, NT], f32)
            nc.gpsimd.dma_start(b_sb[:], brep[:, :])
            magic_sb = constp.tile([128, 1], f32)
            nc.gpsimd.memset(magic_sb[:], MAGIC)

            # persistent dequantized-transposed weight tiles: [k=128, n 4*128]
            # per group of 4 k-tiles; wdqT[nt*NGRP + kt//4][:, (kt%4)*128...]
            wdqT = [
                wdqTp.tile([128, 512], bf16, name=f"wdqT{i}")
                for i in range(NT * NGRP)
            ]

            def x_dma(eng, xb, mb):
                m0, mw = MOFF[mb], MBS[mb]
                for kt in range(KT):
                    if kt % 2 == (0 if eng is nc.sync else 1):
                        eng.dma_start(
                            xb[:, kt * mw : (kt + 1) * mw],
                            x_t[kt * 128 : (kt + 1) * 128, m0 : m0 + mw],
                        )

            # x blocks 0 and 1 pre-issued on both HWDGE queues
            xbs = {}
            for mb in (0, 1):
                xbs[mb] = xp.tile([128, XMAX], bf16, tag="xblk", name=f"xb{mb}")
                x_dma(nc.sync, xbs[mb], mb)
                x_dma(nc.scalar, xbs[mb], mb)

            # weight chunks on the gpsimd (SWDGE) queue
            wns = []
            for nt in range(NT):
                for c in range(K // WCH):
                    wn = wnatp.tile([128, WCH], f32, tag="wnat")
                    nc.gpsimd.dma_start(
                        wn[:], w[nt * 128 : (nt + 1) * 128, c * WCH : (c + 1) * WCH]
                    )
                    wns.append(wn)

            def mm_pass(mb, nt, xb):
                m0, mw = MOFF[mb], MBS[mb]
                acc = accps.tile([128, mw], f32, tag="acc")
                for kt in range(KT):
                    grp = nt * NGRP + kt // 4
                    sub = kt % 4
                    lhsT = wdqT[grp][:, sub * 128 : (sub + 1) * 128]
                    for h in range(mw // 512):
                        nc.tensor.matmul(
                            acc[:, h * 512 : (h + 1) * 512],
                            lhsT,
                            xb[:, kt * mw + h * 512 : kt * mw + h * 512 + 512],
                            start=(kt == 0),
                            stop=(kt == KT - 1),
                        )
                outsb = outp.tile([128, mw], f32, tag="outsb")
                nc.scalar.activation(
                    outsb[:], acc[:], AF.Identity,
                    bias=b_sb[:, nt : nt + 1], scale=1.0,
                )
                nc.scalar.dma_start(
                    out_t[nt * 128 : (nt + 1) * 128, m0 : m0 + mw], outsb[:]
                )

            # ---- phase 0 (dequant+transpose), interleaved with mb0/mb1 MMs
            for nt in range(NT):
                for c in range(K // WCH):
                    wn = wns[nt * (K // WCH) + c]
                    for q in range(WCH // 128):
                        kt = c * (WCH // 128) + q
                        col = nt * G + kt
                        grp = nt * NGRP + kt // 4
                        sub = kt % 4
                        # t1 = (w * (1/s)) + MAGIC on the scalar engine
                        # (its native in*scale+bias form; rounds half-even)
                        t1 = t1p.tile([128, 128], f32)
                        nc.scalar.activation(
                            t1[:],
                            wn[:, q * 128 : (q + 1) * 128],
                            AF.Identity,
                            bias=magic_sb[:],
                            scale=r_sb[:, col : col + 1],
                        )
                        # w_dq = (t1 - MAGIC) * s on DVE (subtract FIRST)
                        wdq = wdqp.tile([128, 128], bf16)
                        nc.vector.tensor_scalar(
                            wdq[:],
                            t1[:],
                            MAGIC,
                            s_sb[:, col : col + 1],
                            op0=OP.subtract,
                            op1=OP.mult,
                        )
                        if sub == 0:
                            ps = tpps.tile([128, 512], bf16)
                        nc.tensor.transpose(
                            ps[:, sub * 128 : (sub + 1) * 128], wdq[:], id_sb[:]
                        )
                        if sub == 3:
                            nc.scalar.copy(wdqT[grp][:], ps[:])
                # row nt ready -> emit block-0/1 passes for this row
                mm_pass(0, nt, xbs[0])
                mm_pass(1, nt, xbs[1])

            # ---- phase 1: stream the remaining m blocks ----
            for mb in range(2, len(MBS)):
                xbs[mb] = xp.tile([128, XMAX], bf16, tag="xblk", name=f"xb{mb}")
                x_dma(nc.sync, xbs[mb], mb)
                x_dma(nc.scalar, xbs[mb], mb)
                for nt in range(NT):
                    mm_pass(mb, nt, xbs[mb])

    _split_waits(nc)
    return nc


def _split_waits(nc, max_waits=1):
    """The walrus build in this container rejects >1 sync-wait per instruction
    ("Too many sync wait commands"). Hoist extra waits onto preceding
    same-engine NOPs, which is semantically identical (in-order engines)."""
    import concourse.mybir as mybir

    for func in nc.m.functions:
        for bb in func.blocks:
            insts = list(bb.instructions)
            new_insts = []
            changed = False
            for inst in insts:
                si = inst.sync_info
                waits = list(si.on_wait) if si is not None and si.on_wait else []
                if len(waits) > max_waits:
                    keep = waits[-max_waits:]
                    for j, wcond in enumerate(waits[:-max_waits]):
                        new_insts.append(
                            mybir.InstNoOp(
                                name=f"{inst.name}-ws{j}",
                                engine=inst.engine,
                                sync_info=mybir.SyncInfo(on_wait=[wcond], on_update=[]),
                            )
                        )
                    si.on_wait = keep
                    inst.sync_info = si
                    changed = True
                new_insts.append(inst)
            if changed:
                bb.instructions = new_insts


def _prep_inputs(x, weight, bias, step_scales):
    x = np.ascontiguousarray(np.asarray(x, dtype=np.float32)).reshape(M, K)
    weight = np.ascontiguousarray(np.asarray(weight, dtype=np.float32))
    bias = np.asarray(bias, dtype=np.float32)
    step_scales = np.asarray(step_scales, dtype=np.float32)

    s_eff = (step_scales + np.float32(EPS)).astype(np.float32)      # [G, N]
    recip = (np.float32(1.0) / s_eff).astype(np.float32)            # [G, N]

    xt = np.asarray(x.T, dtype=BF16)                                # [K, M] bf16
    ident = np.eye(128, dtype=BF16)

    def rep(a):  # [G, NL] -> [128, NT*G] with col nt*G+g = a[g, nt*128+p]
        return np.ascontiguousarray(
            a.T.reshape(NT, 128, G).transpose(1, 0, 2).reshape(128, NT * G)
        )

    in_maps = []
    for c in range(NCORES):
        n0 = c * NL
        sl = slice(n0, n0 + NL)
        in_maps.append(
            {
                "x_t": xt,
                "w": np.ascontiguousarray(weight[sl, :]),
                "srep": rep(s_eff[:, sl]),
                "rrep": rep(recip[:, sl]),
                "brep": np.ascontiguousarray(bias[sl].reshape(NT, 128).T),
                "ident": ident,
            }
        )
    return in_maps


def run_on_hw(x, weight, bias, step_scales, trace=False, **kw):
    from concourse.bass_utils import run_bass_kernel_spmd

    if "nc" not in _NC_CACHE:
        _NC_CACHE["nc"] = _build_nc()
    nc = _NC_CACHE["nc"]
    in_maps = _prep_inputs(x, weight, bias, step_scales)
    res = run_bass_kernel_spmd(
        nc, in_maps, core_ids=list(range(NCORES)), trace=trace, **kw
    )
    out_t = np.concatenate([res.results[c]["out_t"] for c in range(NCORES)], axis=0)
    out = np.ascontiguousarray(out_t.T).reshape(B, S, N)
    return out, res


def kernel(x, weight, bias, step_scales):
    out, _ = run_on_hw(x, weight, bias, step_scales, trace=False)
    return out


# revision 9
# speedup vs baseline: 1.1193x; 1.1193x over previous
# kernel.py — nn_CustomLinearEval: group-dequantized linear layer on 8 trn2 cores.
#
# out[b,s,n] = sum_k x[b,s,k] * w_dq[k,n] + bias[n]
#   w_dq = round(weight.T / s) * s,  s = step_scales[g,n] + 1e-8, g = k // 128
#
# Sharding: column-parallel (tensor-parallel over N). Each core owns 512 of the
# 4096 output features:
#   - DMAs its [512, 4096] fp32 weight shard on the gpsimd (SWDGE) queue so it
#     doesn't contend with the x stream; dequantizes in natural [n, k] layout
#     with round-half-even via the +/-1.5*2^23 magic trick (matching
#     jnp.round): scalar engine does t1 = w*(1/s) + MAGIC (its native
#     multiply-add form, exact), DVE does w_dq = (t1 - MAGIC)*s (subtract
#     FIRST: doing s*t1 - MAGIC*s instead catastrophically cancels).
#     The 128 [n,k] tiles are transposed on the PE once; w_dq^T (bf16, 4 MiB)
#     stays SBUF-resident.
#   - Streams host-pre-transposed bf16 x^T [K, M] in m-blocks of
#     [512,512,512,1024*6,512] columns (double-buffered; chunk DMAs alternate
#     between the sync and scalar HWDGE queues), running back-to-back bf16
#     matmuls (free dim 512) accumulating out^T in PSUM over 32 k-tiles.
#     1024-wide blocks share one LDWEIGHTS per two matmuls.
#   - Block-0 AND block-1 matmul passes are interleaved with the phase-0 rows
#     in PE program order, so the PE has ~14us of matmul work per ~8us
#     dequant row and never idles during warmup.
#   - Bias-add fuses into the PSUM->SBUF copy on the scalar engine; out DMAs
#     issue from the scalar engine's DGE.
# Host gathers the 8 out^T row-shards and transposes once in numpy.

import numpy as np
import ml_dtypes

BF16 = ml_dtypes.bfloat16

GS = 128
EPS = 1e-8
B, S, K, N = 4, 2048, 4096, 4096
M = B * S
NCORES = 8
NL = N // NCORES          # 512 out-features per core
G = K // GS               # 32 quant groups
NT = NL // 128            # 4 n tiles per core
KT = K // 128             # 32 k tiles
MBS = [512] * 16                             # uniform 512-column m-blocks
MOFF = [sum(MBS[:i]) for i in range(len(MBS))]
MAGIC = float(np.float32(12582912.0))  # 1.5 * 2**23: fp32 round-to-nearest-even

_NC_CACHE = {}


def _build_nc():
    import concourse.bass as bass
    import concourse.mybir as mybir
    import concourse.tile as tile

    f32 = mybir.dt.float32
    bf16 = mybir.dt.bfloat16
    AF = mybir.ActivationFunctionType
    OP = mybir.AluOpType

    nc = bass.Bass()
    # x_t: host-pre-transposed x, [K, M] bf16 (pure layout transform on host)
    x_t = nc.dram_tensor("x_t", [K, M], bf16, kind="ExternalInput")
    w = nc.dram_tensor("w", [NL, K], f32, kind="ExternalInput")
    srep = nc.dram_tensor("srep", [128, NT * G], f32, kind="ExternalInput")
    rrep = nc.dram_tensor("rrep", [128, NT * G], f32, kind="ExternalInput")
    brep = nc.dram_tensor("brep", [128, NT], f32, kind="ExternalInput")
    ident = nc.dram_tensor("ident", [128, 128], bf16, kind="ExternalInput")
    out_t = nc.dram_tensor("out_t", [NL, M], f32, kind="ExternalOutput")

    WCH = 2048                # k-columns per weight DMA chunk (16 k-tiles)
    NGRP = KT // 4            # 8 transpose groups of 4 k-tiles per n row
    XMAX = KT * 512           # x block tile columns

    with tile.TileContext(nc) as tc:
        with (
            tc.tile_pool(name="const", bufs=1) as constp,
            tc.tile_pool(name="wdqT", bufs=1) as wdqTp,
            tc.tile_pool(name="xblk", bufs=3) as xp,
            tc.tile_pool(name="wnat", bufs=8) as wnatp,
            tc.tile_pool(name="t1", bufs=4) as t1p,
            tc.tile_pool(name="wdq", bufs=8) as wdqp,
            tc.tile_pool(name="outsb", bufs=3) as outp,
            tc.tile_pool(name="tp_ps", bufs=2, space="PSUM") as tpps,
            tc.tile_pool(name="acc_ps", bufs=3, space="PSUM") as accps,
        ):
            # consts ride the gpsimd queue ahead of the weight chunks
            id_sb = constp.tile([128, 128], bf16)
            nc.gpsimd.dma_start(id_sb[:], ident[:, :])
            s_sb = constp.tile([128, NT * G], f32)
            nc.gpsimd.dma_start(s_sb[:], srep[:, :])
            r_sb = constp.tile([128, NT * G], f32)
            nc.gpsimd.dma_start(r_sb[:], rrep[:, :])
            b_sb = constp.tile([128, NT], f32)
            nc.gpsimd.dma_start(b_sb[:], brep[:, :])
            magic_sb = constp.tile([128, 1], f32)
            nc.gpsimd.memset(magic_sb[:], MAGIC)

            # persistent dequantized-transposed weight tiles: [k=128, n 4*128]
            # per group of 4 k-tiles; wdqT[nt*NGRP + kt//4][:, (kt%4)*128...]
            wdqT = [
                wdqTp.tile([128, 512], bf16, name=f"wdqT{i}")
                for i in range(NT * NGRP)
            ]

            def x_dma(eng, xb, mb):
                m0, mw = MOFF[mb], MBS[mb]
                for kt in range(KT):
                    if kt % 2 == (0 if eng is nc.sync else 1):
                        eng.dma_start(
                            xb[:, kt * mw : (kt + 1) * mw],
                            x_t[kt * 128 : (kt + 1) * 128, m0 : m0 + mw],
                        )

            # weight chunks first: the two chunks of each n-row split across
            # the sync and scalar HWDGE queues so row nt lands at ~nt*6us.
            wns = []
            for nt in range(NT):
                for c in range(K // WCH):
                    wn = wnatp.tile([128, WCH], f32, tag="wnat")
                    eng = nc.sync if c % 2 == 0 else nc.scalar
                    eng.dma_start(
                        wn[:], w[nt * 128 : (nt + 1) * 128, c * WCH : (c + 1) * WCH]
                    )
                    wns.append(wn)

            # x blocks 0-2 pre-issued on both HWDGE queues behind the weights
            xbs = {}
            for mb in (0, 1, 2):
                xbs[mb] = xp.tile([128, XMAX], bf16, tag="xblk", name=f"xb{mb}")
                x_dma(nc.sync, xbs[mb], mb)
                x_dma(nc.scalar, xbs[mb], mb)

            def mm_pass(mb, nt, xb):
                m0, mw = MOFF[mb], MBS[mb]
                acc = accps.tile([128, mw], f32, tag="acc")
                for kt in range(KT):
                    grp = nt * NGRP + kt // 4
                    sub = kt % 4
                    lhsT = wdqT[grp][:, sub * 128 : (sub + 1) * 128]
                    for h in range(mw // 512):
                        nc.tensor.matmul(
                            acc[:, h * 512 : (h + 1) * 512],
                            lhsT,
                            xb[:, kt * mw + h * 512 : kt * mw + h * 512 + 512],
                            start=(kt == 0),
                            stop=(kt == KT - 1),
                        )
                outsb = outp.tile([128, mw], f32, tag="outsb")
                nc.scalar.activation(
                    outsb[:], acc[:], AF.Identity,
                    bias=b_sb[:, nt : nt + 1], scale=1.0,
                )
                nc.scalar.dma_start(
                    out_t[nt * 128 : (nt + 1) * 128, m0 : m0 + mw], outsb[:]
                )

            # ---- phase 0 (dequant+transpose), interleaved with mb0/mb1 MMs
            for nt in range(NT):
                for c in range(K // WCH):
                    wn = wns[nt * (K // WCH) + c]
                    for q in range(WCH // 128):
                        kt = c * (WCH // 128) + q
                        col = nt * G + kt
                        grp = nt * NGRP + kt // 4
                        sub = kt % 4
                        # t1 = (w * (1/s)) + MAGIC on the scalar engine
                        # (its native in*scale+bias form; rounds half-even)
                        t1 = t1p.tile([128, 128], f32)
                        nc.scalar.activation(
                            t1[:],
                            wn[:, q * 128 : (q + 1) * 128],
                            AF.Identity,
                            bias=magic_sb[:],
                            scale=r_sb[:, col : col + 1],
                        )
                        # w_dq = (t1 - MAGIC) * s on DVE (subtract FIRST)
                        wdq = wdqp.tile([128, 128], bf16)
                        nc.vector.tensor_scalar(
                            wdq[:],
                            t1[:],
                            MAGIC,
                            s_sb[:, col : col + 1],
                            op0=OP.subtract,
                            op1=OP.mult,
                        )
                        if sub == 0:
                            ps = tpps.tile([128, 512], bf16)
                        nc.tensor.transpose(
                            ps[:, sub * 128 : (sub + 1) * 128], wdq[:], id_sb[:]
                        )
                        if sub == 3:
                            nc.scalar.copy(wdqT[grp][:], ps[:])
                # row nt ready -> emit block-0/1 passes for this row
                mm_pass(0, nt, xbs[0])
                mm_pass(1, nt, xbs[1])

            # ---- phase 1: stream the remaining m blocks ----
            for mb in range(2, len(MBS)):
                nxt = mb + 1
                if nxt < len(MBS):
                    xbs[nxt] = xp.tile([128, XMAX], bf16, tag="xblk", name=f"xb{nxt}")
                    x_dma(nc.sync, xbs[nxt], nxt)
                    x_dma(nc.scalar, xbs[nxt], nxt)
                for nt in range(NT):
                    mm_pass(mb, nt, xbs[mb])

    _split_waits(nc)
    return nc


def _split_waits(nc, max_waits=1):
    """The walrus build in this container rejects >1 sync-wait per instruction
    ("Too many sync wait commands"). Hoist extra waits onto preceding
    same-engine NOPs, which is semantically identical (in-order engines)."""
    import concourse.mybir as mybir

    for func in nc.m.functions:
        for bb in func.blocks:
            insts = list(bb.instructions)
            new_insts = []
            changed = False
            for inst in insts:
                si = inst.sync_info
                waits = list(si.on_wait) if si is not None and si.on_wait else []
                if len(waits) > max_waits:
                    keep = waits[-max_waits:]
                    for j, wcond in enumerate(waits[:-max_waits]):
                        new_insts.append(
                            mybir.InstNoOp(
                                name=f"{inst.name}-ws{j}",
                                engine=inst.engine,
                                sync_info=mybir.SyncInfo(on_wait=[wcond], on_update=[]),
                            )
                        )
                    si.on_wait = keep
                    inst.sync_info = si
                    changed = True
                new_insts.append(inst)
            if changed:
                bb.instructions = new_insts


def _prep_inputs(x, weight, bias, step_scales):
    x = np.ascontiguousarray(np.asarray(x, dtype=np.float32)).reshape(M, K)
    weight = np.ascontiguousarray(np.asarray(weight, dtype=np.float32))
    bias = np.asarray(bias, dtype=np.float32)
    step_scales = np.asarray(step_scales, dtype=np.float32)

    s_eff = (step_scales + np.float32(EPS)).astype(np.float32)      # [G, N]
    recip = (np.float32(1.0) / s_eff).astype(np.float32)            # [G, N]

    xt = np.asarray(x.T, dtype=BF16)                                # [K, M] bf16
    ident = np.eye(128, dtype=BF16)

    def rep(a):  # [G, NL] -> [128, NT*G] with col nt*G+g = a[g, nt*128+p]
        return np.ascontiguousarray(
            a.T.reshape(NT, 128, G).transpose(1, 0, 2).reshape(128, NT * G)
        )

    in_maps = []
    for c in range(NCORES):
        n0 = c * NL
        sl = slice(n0, n0 + NL)
        in_maps.append(
            {
                "x_t": xt,
                "w": np.ascontiguousarray(weight[sl, :]),
                "srep": rep(s_eff[:, sl]),
                "rrep": rep(recip[:, sl]),
                "brep": np.ascontiguousarray(bias[sl].reshape(NT, 128).T),
                "ident": ident,
            }
        )
    return in_maps


def run_on_hw(x, weight, bias, step_scales, trace=False, **kw):
    from concourse.bass_utils import run_bass_kernel_spmd

    if "nc" not in _NC_CACHE:
        _NC_CACHE["nc"] = _build_nc()
    nc = _NC_CACHE["nc"]
    in_maps = _prep_inputs(x, weight, bias, step_scales)
    res = run_bass_kernel_spmd(
        nc, in_maps, core_ids=list(range(NCORES)), trace=trace, **kw
    )
    out_t = np.concatenate([res.results[c]["out_t"] for c in range(NCORES)], axis=0)
    out = np.ascontiguousarray(out_t.T).reshape(B, S, N)
    return out, res


def kernel(x, weight, bias, step_scales):
    out, _ = run_on_hw(x, weight, bias, step_scales, trace=False)
    return out
